# revision 2
# baseline (speedup 1.0000x reference)
"""TRN2 Bass kernel for nn_MindieCifTimestamp (CIF no-hidden scan).

reference:  acc' = acc + a_t;  fire_t = acc';  acc' -= (acc' >= 1.0)
i.e. mod-1 cumulative summation per batch row.

Strategy (data-parallel over 8 NeuronCores, 512 rows each):
  per 128-row group, streamed in column blocks:
    - chunked cumsum L (chunk C=64) via the DVE tensor_tensor_scan
    - chunk bases c_k in [0,1) via a hierarchical frac chain on chunk tails
    - X = L + c_k (unwrapped accumulator), G = floor(X) (ScalarE int32 cast),
      R = X - G = frac(X)
    - fire_t = R_{t-1} + a_t (R_{-1} = integrate; R at a chunk tail equals
      the next chunk's base so no boundary fixups)
    - integrate_new = R_{T-1}
"""
import os
_p = os.environ.get("JAX_PLATFORMS", "")
if "axon" not in _p.split(","):
    os.environ["JAX_PLATFORMS"] = "axon,cpu" if not _p else _p + ",axon"

import numpy as np

import concourse.bass as bass
import concourse.mybir as mybir
import concourse.tile as tile
from concourse.bass_utils import run_bass_kernel_spmd

F32 = mybir.dt.float32
I32 = mybir.dt.int32

B, T = 4096, 8192
NCORES = 8
ROWS = B // NCORES            # 512
W = 2048                      # column block
C = 64                        # cumsum chunk

X_MODE = "step0"
GF_ON_ACT = False
R_ON_GPSIMD = True
FIRE_ON_GPSIMD = False


# ---------------------------------------------------------------- tile patch
# This walrus build encodes at most ONE sync-wait per instruction; Tile's
# scheduler freely attaches several. Redistribute: one-wait nops ahead of
# any multi-wait instruction, and a bare kernel-tail drain.
def _patched_drain_and_barrier(self, tick_clock, wait_clock):
    from bass_rust import ScopedClock
    nc = self.nc
    probe = nc.sync.nop()
    wait_clock.add_sem_waits(probe.ins, ScopedClock({None: tick_clock.global_clock}))
    si = probe.ins.sync_info
    waits = list(si.on_wait or []) if si is not None else []
    if len(waits) > 1:
        si.on_wait = waits[:1]
        for w in waits[1:]:
            extra = nc.sync.nop()
            esi = extra.ins.sync_info
            if esi is None:
                extra.ins.sync_info = type(si)(on_wait=[w], on_update=[])
            else:
                esi.on_wait = [w]
    nc.sync.drain()
    nc.all_engine_barrier()
    assert self.sems is not None
    popped = nc._tile_sem_poison_stack.pop()
    assert popped is self._sem_poison
    nc.clear_and_free_semaphores(list(self.sems.allocated().values()))
    nc.all_engine_barrier()


tile.TileContext._drain_and_barrier = _patched_drain_and_barrier


def _split_multi_waits(nc):
    import concourse.mybir as mybir
    seen = set()
    for name, bassbb in nc.bb_map.items():
        bb = bassbb.bb if hasattr(bassbb, "bb") else bassbb
        if id(bb) in seen:
            continue
        seen.add(id(bb))
        insts = list(bb.instructions)
        out = []
        changed = False
        for inst in insts:
            si = inst.sync_info
            waits = list(si.on_wait or []) if si is not None else []
            if len(waits) > 1:
                changed = True
                for w in waits[:-1]:
                    nop = mybir.InstNoOp(name=nc.get_next_instruction_name())
                    nop.engine = inst.engine
                    nop.sync_info = type(si)(on_wait=[w], on_update=[])
                    nc.register_instruction(nop)
                    out.append(nop)
                si.on_wait = waits[-1:]
            out.append(inst)
        if changed:
            bb.instructions = out


# ---------------------------------------------------------------- builder
def _expand_ap(ap, reps):
    (ps, pc), (fs, fc) = ap.ap
    return bass.AP(ap.tensor, ap.offset, [[ps, pc], [fs, fc], [0, reps]])


def _build(n_rows, t_len, w, c_len):
    G_ROWS = n_rows // 128
    NB = t_len // w
    K = w // c_len

    nc = bass.Bass()
    alphas = nc.dram_tensor("us_alphas", [n_rows, t_len], F32, kind="ExternalInput")
    integ = nc.dram_tensor("integrate", [n_rows], F32, kind="ExternalInput")
    fires = nc.dram_tensor("fires", [n_rows, t_len], F32, kind="ExternalOutput")
    integ_new = nc.dram_tensor("integ_new", [n_rows], F32, kind="ExternalOutput")

    integ2d = integ[:].rearrange("(g p o) -> g p o", g=G_ROWS, o=1)
    inew2d = integ_new[:].rearrange("(g p o) -> g p o", g=G_ROWS, o=1)

    with tile.TileContext(nc) as tc:
        with (
            tc.tile_pool(name="big", bufs=3) as big,
            tc.tile_pool(name="small", bufs=2) as small,
            tc.tile_pool(name="consts", bufs=1) as consts,
        ):
            neghalf = consts.tile([128, 1], F32)
            nc.vector.memset(neghalf[:, :], -0.5)

            for g in range(G_ROWS):
                rtail = small.tile([128, 1], F32, name=f"rt_{g}", tag="rtail")
                nc.sync.dma_start(out=rtail[:, :], in_=integ2d[g])
                base = small.tile([128, 1], F32, name=f"ba_{g}", tag="base")
                nc.vector.tensor_copy(base[:, :], rtail[:, :])

                for bb_ in range(NB):
                    a = big.tile([128, w], F32, name=f"a_{g}_{bb_}", tag="a")
                    nc.sync.dma_start(
                        out=a[:, :],
                        in_=alphas[g * 128:(g + 1) * 128, bb_ * w:(bb_ + 1) * w])

                    l = big.tile([128, w], F32, name=f"l_{g}_{bb_}", tag="l")
                    for k in range(K):
                        s_ = slice(k * c_len, (k + 1) * c_len)
                        nc.vector.tensor_tensor_scan(
                            out=l[:, s_], data0=a[:, s_], data1=a[:, s_],
                            initial=0.0,
                            op0=mybir.AluOpType.add, op1=mybir.AluOpType.bypass)

                    tails = small.tile([128, K], F32, name=f"tl_{g}_{bb_}", tag="tails")
                    nc.vector.tensor_copy(tails[:, :], l[:, c_len - 1::c_len])
                    ti = small.tile([128, K], I32, name=f"ti_{g}_{bb_}", tag="ti")
                    nc.vector.tensor_scalar(
                        out=ti[:, :], in0=tails[:, :], scalar1=0.5, scalar2=None,
                        op0=mybir.AluOpType.subtract)
                    tf = small.tile([128, K], F32, name=f"tf_{g}_{bb_}", tag="tf")
                    nc.vector.tensor_copy(tf[:, :], ti[:, :])
                    ft = small.tile([128, K], F32, name=f"ftl_{g}_{bb_}", tag="ft")
                    nc.vector.tensor_sub(ft[:, :], tails[:, :], tf[:, :])
                    p2 = small.tile([128, K], F32, name=f"p2_{g}_{bb_}", tag="p2")
                    nc.vector.tensor_tensor_scan(
                        out=p2[:, :], data0=ft[:, :], data1=ft[:, :], initial=0.0,
                        op0=mybir.AluOpType.add, op1=mybir.AluOpType.bypass)
                    cpre = small.tile([128, K], F32, name=f"cp_{g}_{bb_}", tag="cpre")
                    nc.vector.tensor_copy(cpre[:, 0:1], base[:, :])
                    if K > 1:
                        nc.vector.tensor_scalar(
                            out=cpre[:, 1:K], in0=p2[:, 0:K - 1],
                            scalar1=base[:, 0:1], scalar2=None,
                            op0=mybir.AluOpType.add)
                    ci = small.tile([128, K], I32, name=f"ci_{g}_{bb_}", tag="ci")
                    nc.vector.tensor_scalar(
                        out=ci[:, :], in0=cpre[:, :], scalar1=0.5, scalar2=None,
                        op0=mybir.AluOpType.subtract)
                    cf = small.tile([128, K], F32, name=f"cf_{g}_{bb_}", tag="cf")
                    nc.vector.tensor_copy(cf[:, :], ci[:, :])
                    c = small.tile([128, K], F32, name=f"c_{g}_{bb_}", tag="c")
                    nc.vector.tensor_sub(c[:, :], cpre[:, :], cf[:, :])

                    if bb_ < NB - 1:
                        nb_ = small.tile([128, 1], F32, name=f"nb_{g}_{bb_}", tag="nb")
                        nc.vector.tensor_add(nb_[:, :], p2[:, K - 1:K], base[:, :])
                        nbi = small.tile([128, 1], I32, name=f"ni_{g}_{bb_}", tag="nbi")
                        nc.vector.tensor_scalar(
                            out=nbi[:, :], in0=nb_[:, :], scalar1=0.5, scalar2=None,
                            op0=mybir.AluOpType.subtract)
                        nbf = small.tile([128, 1], F32, name=f"nf_{g}_{bb_}", tag="nbf")
                        nc.vector.tensor_copy(nbf[:, :], nbi[:, :])
                        base = small.tile([128, 1], F32, name=f"ba_{g}_{bb_}",
                                          tag="base")
                        nc.vector.tensor_sub(base[:, :], nb_[:, :], nbf[:, :])

                    x = big.tile([128, w], F32, name=f"x_{g}_{bb_}", tag="x")
                    if X_MODE == "step0":
                        c3 = _expand_ap(c[:, :], c_len)
                        l3 = l[:, :].rearrange("p (k c) -> p k c", c=c_len)
                        x3 = x[:, :].rearrange("p (k c) -> p k c", c=c_len)
                        nc.vector.tensor_tensor(
                            out=x3, in0=l3, in1=c3, op=mybir.AluOpType.add)
                    else:
                        for k in range(K):
                            s_ = slice(k * c_len, (k + 1) * c_len)
                            nc.vector.tensor_scalar(
                                out=x[:, s_], in0=l[:, s_],
                                scalar1=c[:, k:k + 1], scalar2=None,
                                op0=mybir.AluOpType.add)

                    gi = big.tile([128, w], I32, name=f"gi_{g}_{bb_}", tag="gi")
                    nc.scalar.activation(
                        gi[:, :], x[:, :], mybir.ActivationFunctionType.Identity,
                        bias=neghalf[:, 0:1], scale=1.0)
                    gf = big.tile([128, w], F32, name=f"gf_{g}_{bb_}", tag="gf")
                    if GF_ON_ACT:
                        nc.scalar.activation(
                            gf[:, :], gi[:, :], mybir.ActivationFunctionType.Copy)
                    else:
                        nc.vector.tensor_copy(gf[:, :], gi[:, :])

                    r = big.tile([128, w], F32, name=f"r_{g}_{bb_}", tag="r")
                    eng_r = nc.gpsimd if R_ON_GPSIMD else nc.vector
                    eng_r.tensor_sub(r[:, :], x[:, :], gf[:, :])

                    f = big.tile([128, w], F32, name=f"f_{g}_{bb_}", tag="f")
                    eng_f = nc.gpsimd if FIRE_ON_GPSIMD else nc.vector
                    eng_f.tensor_add(f[:, 0:1], rtail[:, :], a[:, 0:1])
                    eng_f.tensor_add(f[:, 1:w], r[:, 0:w - 1], a[:, 1:w])

                    rtail = small.tile([128, 1], F32, name=f"rt_{g}_{bb_}",
                                       tag="rtail")
                    nc.vector.tensor_copy(rtail[:, :], r[:, w - 1:w])

                    nc.sync.dma_start(
                        out=fires[g * 128:(g + 1) * 128, bb_ * w:(bb_ + 1) * w],
                        in_=f[:, :])

                nc.sync.dma_start(out=inew2d[g], in_=rtail[:, :])

    _split_multi_waits(nc)
    return nc


_CACHED = {}


def _get_nc():
    key = (ROWS, T, W, C, X_MODE, GF_ON_ACT, R_ON_GPSIMD, FIRE_ON_GPSIMD)
    if key not in _CACHED:
        _CACHED[key] = _build(ROWS, T, W, C)
    return _CACHED[key]


def kernel(us_alphas, integrate, _want_results_obj=False, _trace=False):
    us_alphas = np.ascontiguousarray(np.asarray(us_alphas, dtype=np.float32))
    integrate = np.ascontiguousarray(np.asarray(integrate, dtype=np.float32))
    assert us_alphas.shape == (B, T) and integrate.shape == (B,)

    nc = _get_nc()
    in_maps = [
        {
            "us_alphas": us_alphas[i * ROWS:(i + 1) * ROWS],
            "integrate": integrate[i * ROWS:(i + 1) * ROWS],
        }
        for i in range(NCORES)
    ]
    res = run_bass_kernel_spmd(nc, in_maps, list(range(NCORES)), trace=_trace)

    fires = np.concatenate([res.results[i]["fires"] for i in range(NCORES)], axis=0)
    integ_new = np.concatenate(
        [res.results[i]["integ_new"] for i in range(NCORES)], axis=0)
    if _want_results_obj:
        return (fires, integ_new), res
    return fires, integ_new
